# revision 11
# baseline (speedup 1.0000x reference)
"""A3TGCN (2-period TGCN GRU over a fixed 64-node graph) on 8 TRN2 NeuronCores.

Strategy: pure data parallel over the batch (256 samples/core). Per core:
  - Embedding rows are fetched via descriptor gather from a fused table
    [Tz | Th | E | pad] (bf16, 512B rows), where Tz = emb_table @ (W_z @ Lz1),
    Th likewise, E = raw table. Layout out of the gather ("N2"):
    partition = (s%2)*64 + node, free = (pair, 64 elems).
  - Period-1 gate pre-activations via PE matmuls with block-diagonal
    normalized-adjacency stationaries (2 samples per 128-wide matmul).
  - Period 2 via D2 = A2 @ E, transposed per-sample to "T2" layout
    (partition = (half, feature)), then feature-space matmuls; the
    GRU L2-matmuls accumulate into the same PSUM banks.
  - Gates on ACT (sigmoid/tanh), elementwise on DVE, pooling via
    free-dim segmented reduces, classifier as two tiny matmuls.
"""

import os
import re
import sys

import numpy as np

for _p in ("/opt/trn_rl_repo",):
    if os.path.isdir(_p) and _p not in sys.path:
        sys.path.insert(0, _p)

import ml_dtypes  # noqa: E402

import concourse.bass as bass  # noqa: E402
import concourse.tile as tile  # noqa: E402
from concourse import mybir  # noqa: E402
from concourse.bass_utils import run_bass_kernel_spmd  # noqa: E402
from concourse.masks import make_identity  # noqa: E402
from concourse.vector_clock import ScopedClock, VectorClock  # noqa: E402

BF16 = ml_dtypes.bfloat16
F32 = mybir.dt.float32
BF = mybir.dt.bfloat16
I16 = mybir.dt.int16

B, NCOLS, CARD, EMB, HID, NN = 2048, 64, 128, 64, 64, 64
NCORES = 8
S = B // NCORES            # samples per core (256)
G = 4                      # sample groups per core
SPG = S // G               # samples per group (64)
QPG = SPG // 2             # pairs per group (32)
NCHUNK = 4                 # chunks per group
PPC = QPG // NCHUNK        # pairs per chunk (8)
APG = QPG // 2             # pair-pairs ("a" index) per group (16)
APC = APG // NCHUNK        # pair-pairs per chunk (4)

# gather modes
HOST_GATHER = os.environ.get("K_HOST_GATHER", "0") == "1"
PE_TRANSPOSE = os.environ.get("K_PE_TRANSPOSE", "0") == "1"


_ENG_MAX_WAITS = int(os.environ.get("K_MAX_WAITS", "1"))


class SplitDrainTileContext(tile.TileContext):
    """This neuronxcc build rejects instructions carrying more than one sync
    wait ("Too many sync wait commands"). Post-process the scheduled program:
    move extra waits onto dedicated NOPs inserted just before the owning
    instruction, and emit the tail drain as a series of 1-wait drains."""

    def _eng(self, engine):
        nc = self.nc
        m = {
            mybir.EngineType.SP: nc.sync,
            mybir.EngineType.Activation: nc.scalar,
            mybir.EngineType.DVE: nc.vector,
            mybir.EngineType.PE: nc.tensor,
            mybir.EngineType.Pool: nc.gpsimd,
        }
        return m[engine]

    def _split_waits(self):
        nc = self.nc
        lim = _ENG_MAX_WAITS
        cur_list = nc.cur_bb.bb
        for f in nc.m.functions:
            for bb in f.blocks:
                insts = list(bb.instructions)
                if not any(
                    i.sync_info is not None and len(i.sync_info.on_wait) > lim
                    for i in insts
                ):
                    continue
                new = []
                for inst in insts:
                    si = inst.sync_info
                    if si is not None and len(si.on_wait) > lim:
                        waits = list(si.on_wait)
                        extras, keep = waits[:-lim], waits[-lim:]
                        for w in extras:
                            nop = self._eng(inst.engine).nop()
                            nopi = getattr(nop, "ins", nop)
                            # un-append from the current block
                            tail = list(cur_list.instructions)
                            assert tail[-1].name == nopi.name
                            cur_list.instructions = tail[:-1]
                            nopi.sync_info = mybir.SyncInfo(on_wait=[w],
                                                            on_update=[])
                            new.append(nopi)
                        si.on_wait = keep
                    new.append(inst)
                bb.instructions = new

    def _drain_and_barrier(self, tick_clock, wait_clock):
        self._split_waits()
        gc = tick_clock.global_clock
        ticks = list(map(int, re.findall(r"\d+", repr(gc))))
        nz = [(p, t) for p, t in enumerate(ticks) if t > 0]
        emitted = False
        for p, t in nz:
            pv = VectorClock()
            for _ in range(t):
                pv.advance(p)
            d = self.nc.sync.drain()
            wait_clock.add_sem_waits(d.ins, ScopedClock({None: pv}))
            emitted = True
        if not emitted:
            self.nc.sync.drain()
        self.nc.all_engine_barrier()
        popped = self.nc._tile_sem_poison_stack.pop()
        assert popped is self._sem_poison
        self.nc.clear_and_free_semaphores(list(self.sems.allocated().values()))
        self.nc.all_engine_barrier()


def _blockdiag(m):
    z = np.zeros_like(m)
    return np.block([[m, z], [z, m]])


def _wrap16(seg):
    """Wrap a flat index list into the [16, n/16] partition layout the
    SWDGE gather consumes (element i at partition i%16, slot i//16),
    replicated to all 128 partitions."""
    n = seg.shape[0]
    w = seg.reshape(n // 16, 16).T.astype(np.int16)  # [16, n/16]
    return np.tile(w, (8, 1))                         # [128, n/16]


def host_prep(inputs):
    """All O(params)+O(E) preprocessing. Returns per-core in_maps plus the
    compile-time scalars."""
    f = lambda k: np.asarray(inputs[k], np.float32)
    xb = np.asarray(inputs["x_batch"]).astype(np.int64)
    los = np.asarray(inputs["LOS_batch"]).astype(np.int64)
    ei = np.asarray(inputs["template_edge_index"]).astype(np.int64)
    emb = f("emb_table")

    for bk in ("b_z", "Lz_b", "b_r", "Lr_b", "b_h", "Lh_b", "cls_b1", "cls_b2"):
        assert np.allclose(f(bk), 0.0), f"nonzero bias {bk} unsupported"

    x = np.concatenate([xb, los[:, None]], axis=1)              # [B, 64]
    gidx = (x + np.arange(NCOLS, dtype=np.int64)[None, :] * CARD).astype(np.int64)

    # normalized adjacency with self loops: A1[t, n]; A2 = A1 @ P (col flip)
    src = np.concatenate([ei[0], np.arange(NN)])
    tgt = np.concatenate([ei[1], np.arange(NN)])
    deg = np.zeros(NN, np.float32)
    np.add.at(deg, tgt, 1.0)
    dinv = np.where(deg > 0, deg**-0.5, 0.0)
    norm = dinv[src] * dinv[tgt]
    A1 = np.zeros((NN, NN), np.float32)
    np.add.at(A1, (tgt, src), norm)
    A2 = A1[:, ::-1].copy()

    Mz = f("W_z") @ f("Lz_W")[:HID]
    Mr = f("W_r") @ f("Lr_W")[:HID]
    Mh = f("W_h") @ f("Lh_W")[:HID]
    Lz2 = f("Lz_W")[HID:]
    Lr2 = f("Lr_W")[HID:]
    Lh2 = f("Lh_W")[HID:]
    Tz = emb @ Mz
    Th = emb @ Mh

    table = np.zeros((NCOLS * CARD, 256), np.float32)
    table[:, 0:64] = Tz
    table[:, 64:128] = Th
    table[:, 128:192] = emb
    table_bf = table.astype(BF16)

    attn = f("attn")
    e = np.exp(attn - attn.max())
    p = e / e.sum()
    a1, a2 = float(p[0] / NN), float(p[1] / NN)

    consts = {
        "bdA1": _blockdiag(A1.T).astype(BF16),
        "bdA2": _blockdiag(A2.T).astype(BF16),
        "bdMz": _blockdiag(Mz).astype(BF16),
        "bdMr": _blockdiag(Mr).astype(BF16),
        "bdMh": _blockdiag(Mh).astype(BF16),
        "bdLz2": _blockdiag(Lz2).astype(BF16),
        "bdLr2": _blockdiag(Lr2).astype(BF16),
        "bdLh2": _blockdiag(Lh2).astype(BF16),
        "w1": np.concatenate([f("cls_W1"), f("cls_W1")], 0).astype(BF16),
        "w2": f("cls_W2").astype(BF16),                 # [128, 1]
        "table": table_bf,
    }
    # static transpose-gather index list: i = (a, t) -> idx = a*64 + t (iota)
    consts["tgidx"] = _wrap16(np.arange(APG * NN, dtype=np.int64))  # [128, 64]

    in_maps = []
    for c in range(NCORES):
        core_idx = gidx[c * S : (c + 1) * S]            # [256, 64]
        flat = core_idx.reshape(-1)                     # j = s_local*64 + n
        m = dict(consts)
        m["midx"] = np.concatenate(
            [_wrap16(flat[g * 4096 : (g + 1) * 4096]) for g in range(G)], axis=1
        )                                               # [128, 1024]
        if HOST_GATHER:
            rows = table_bf[flat]                       # [16384, 256]
            m["ebuf"] = np.ascontiguousarray(
                rows.reshape(128, 128, 256).transpose(1, 0, 2)
            )                                           # [part, pair, elem]
        in_maps.append(m)
    return in_maps, a1, a2


def build(a1, a2):
    nc = bass.Bass("TRN2", target_bir_lowering=False, debug=False,
                   num_devices=NCORES)
    dp = lambda name, shape, dt: nc.declare_dram_parameter(name, shape, dt,
                                                           isOutput=False)
    cd = {}
    for nm in ("bdA1", "bdA2", "bdMz", "bdMr", "bdMh", "bdLz2", "bdLr2",
               "bdLh2"):
        cd[nm] = dp(nm, [128, 128], BF)
    cd["w1"] = dp("w1", [128, 128], BF)
    cd["w2"] = dp("w2", [128, 1], BF)
    cd["table"] = dp("table", [NCOLS * CARD, 256], BF)
    cd["tgidx"] = dp("tgidx", [128, 64], I16)
    cd["midx"] = dp("midx", [128, 1024], I16)
    if HOST_GATHER:
        cd["ebuf"] = dp("ebuf", [128, 128, 256], BF)
    out_d = nc.declare_dram_parameter("out", [S], F32, isOutput=True)

    from contextlib import ExitStack

    with SplitDrainTileContext(nc) as tc, ExitStack() as ctx:
        sing = ctx.enter_context(tc.tile_pool(name="sing", bufs=1))
        ppool = ctx.enter_context(tc.tile_pool(name="ppool", bufs=1))
        loop_ctx = ctx.enter_context(ExitStack())
        epool = loop_ctx.enter_context(tc.tile_pool(name="epool", bufs=2))
        gpool = loop_ctx.enter_context(tc.tile_pool(name="gpool", bufs=2))
        cpool = loop_ctx.enter_context(tc.tile_pool(name="cpool", bufs=3))
        ps_p1 = loop_ctx.enter_context(
            tc.tile_pool(name="ps_p1", bufs=1, space="PSUM"))
        ps_p2 = loop_ctx.enter_context(
            tc.tile_pool(name="ps_p2", bufs=1, space="PSUM"))
        ps_tp = loop_ctx.enter_context(
            tc.tile_pool(name="ps_tp", bufs=1, space="PSUM"))

        # constants
        ct = {}
        for nm in ("bdA1", "bdA2", "bdMz", "bdMr", "bdMh", "bdLz2", "bdLr2",
                   "bdLh2"):
            ct[nm] = sing.tile([128, 128], BF, tag=nm, name=nm)
            nc.sync.dma_start(out=ct[nm], in_=cd[nm][:, :])
        w1_t = sing.tile([128, 128], BF, tag="w1")
        nc.sync.dma_start(out=w1_t, in_=cd["w1"][:, :])
        w2_t = sing.tile([128, 1], BF, tag="w2")
        nc.sync.dma_start(out=w2_t, in_=cd["w2"][:, :])
        midx_t = sing.tile([128, 1024], I16, tag="midx")
        nc.sync.dma_start(out=midx_t, in_=cd["midx"][:, :])
        tgidx_t = sing.tile([128, 64], I16, tag="tgidx")
        nc.sync.dma_start(out=tgidx_t, in_=cd["tgidx"][:, :])
        identity = sing.tile([128, 128], BF, tag="ident")
        make_identity(nc, identity)

        pool1 = ppool.tile([128, G, 2, APG], F32, tag="pool1")
        pool2 = ppool.tile([128, G, 2, APG], F32, tag="pool2")
        outbuf = ppool.tile([1, G * APG * 4], F32, tag="outbuf")  # (g,a,u,j)

        Sg = mybir.ActivationFunctionType.Sigmoid
        Tn = mybir.ActivationFunctionType.Tanh
        Mu = mybir.AluOpType.mult
        Ad = mybir.AluOpType.add
        Sb = mybir.AluOpType.subtract

        for g in range(G):
            ebuf = epool.tile([128, QPG, 256], BF, tag="ebuf")
            if HOST_GATHER:
                nc.sync.dma_start(out=ebuf,
                                  in_=cd["ebuf"][:, g * QPG:(g + 1) * QPG, :])
            else:
                nc.gpsimd.dma_gather(
                    out_ap=ebuf,
                    in_ap=cd["table"][:, :],
                    idxs_ap=midx_t[:, g * 256:(g + 1) * 256],
                    num_idxs=4096,
                    num_idxs_reg=4096,
                    elem_size=256,
                    queue_num=g % 2,
                )
            h1 = gpool.tile([128, QPG, HID], BF, tag="h1")
            d2c = gpool.tile([128, QPG, HID], BF, tag="d2c")

            for k in range(NCHUNK):
                ps = slice(k * PPC, (k + 1) * PPC)
                c1z = ps_p1.tile([128, PPC, HID], F32, tag="c1z")
                nc.tensor.matmul(c1z, ct["bdA1"], ebuf[:, ps, 0:64],
                                 start=True, stop=True)
                c1h = ps_p1.tile([128, PPC, HID], F32, tag="c1h")
                nc.tensor.matmul(c1h, ct["bdA1"], ebuf[:, ps, 64:128],
                                 start=True, stop=True)
                d2p = ps_p1.tile([128, PPC, HID], F32, tag="d2p")
                nc.tensor.matmul(d2p, ct["bdA2"], ebuf[:, ps, 128:192],
                                 start=True, stop=True)
                z1c = cpool.tile([128, PPC, HID], BF, tag="z1c")
                nc.scalar.activation(z1c, c1z, Sg, scale=-1.0)
                ht1 = cpool.tile([128, PPC, HID], BF, tag="ht1")
                nc.scalar.activation(ht1, c1h, Tn)
                nc.vector.tensor_tensor(out=h1[:, ps, :], in0=z1c, in1=ht1,
                                        op=Mu)
                nc.vector.tensor_copy(out=d2c[:, ps, :], in_=d2p)

            # per-sample 64x64 transpose: N2 -> T2 (partition (u,h),
            # free (j, a, t)); sample s = 64 g + 4 a + 2 u + j
            h1t = gpool.tile([128, 2, APG, NN], BF, tag="h1t")
            d2t = gpool.tile([128, 2, APG, NN], BF, tag="d2t")
            if PE_TRANSPOSE:
                for src_t, dst_t in ((h1, h1t), (d2c, d2t)):
                    for half in range(2):
                        tp = ps_tp.tile([128, 2, APG // 2, NN], BF, tag="tp")
                        for ml in range(APG // 2):
                            a = half * (APG // 2) + ml
                            nc.tensor.transpose(tp[:, :, ml, :],
                                                src_t[:, 2 * a:2 * a + 2, :],
                                                identity)
                        nc.vector.tensor_copy(
                            out=dst_t[:, :,
                                      half * (APG // 2):(half + 1) * (APG // 2),
                                      :],
                            in_=tp)
            else:
                for src_t, dst_t in ((h1, h1t), (d2c, d2t)):
                    nc.gpsimd.dma_gather(
                        out_ap=dst_t,
                        in_ap=src_t,
                        idxs_ap=tgidx_t,
                        num_idxs=APG * NN,
                        num_idxs_reg=APG * NN,
                        elem_size=256,
                        transpose=True,
                        queue_num=2 + (g % 2),
                        sbuf_tokens_per_rank=64,
                        sbuf_free_dim_per_rank=256,
                    )

            for k in range(NCHUNK):
                asl = slice(k * APC, (k + 1) * APC)
                h1s = h1t[:, :, asl, :]
                c2z = ps_p2.tile([128, 2, APC, HID], F32, tag="c2z")
                nc.tensor.matmul(c2z, ct["bdMz"], d2t[:, :, asl, :],
                                 start=True, stop=False)
                nc.tensor.matmul(c2z, ct["bdLz2"], h1s, start=False, stop=True)
                c2r = ps_p2.tile([128, 2, APC, HID], F32, tag="c2r")
                nc.tensor.matmul(c2r, ct["bdMr"], d2t[:, :, asl, :],
                                 start=True, stop=False)
                nc.tensor.matmul(c2r, ct["bdLr2"], h1s, start=False, stop=True)
                z2 = cpool.tile([128, 2, APC, HID], BF, tag="z2")
                nc.scalar.activation(z2, c2z, Sg)
                r2 = cpool.tile([128, 2, APC, HID], BF, tag="r2")
                nc.scalar.activation(r2, c2r, Sg)
                v = cpool.tile([128, 2, APC, HID], BF, tag="v")
                nc.vector.tensor_tensor(out=v, in0=h1s, in1=r2, op=Mu)
                c2h = ps_p2.tile([128, 2, APC, HID], F32, tag="c2h")
                nc.tensor.matmul(c2h, ct["bdMh"], d2t[:, :, asl, :],
                                 start=True, stop=False)
                nc.tensor.matmul(c2h, ct["bdLh2"], v, start=False, stop=True)
                ht2 = cpool.tile([128, 2, APC, HID], BF, tag="ht2")
                nc.scalar.activation(ht2, c2h, Tn)
                dd = cpool.tile([128, 2, APC, HID], BF, tag="dd")
                nc.vector.tensor_tensor(out=dd, in0=h1s, in1=ht2, op=Sb)
                mm = cpool.tile([128, 2, APC, HID], BF, tag="mm")
                nc.vector.tensor_tensor(out=mm, in0=z2, in1=dd, op=Mu)
                h2 = cpool.tile([128, 2, APC, HID], BF, tag="h2")
                nc.vector.tensor_tensor(out=h2, in0=ht2, in1=mm, op=Ad)
                nc.vector.tensor_reduce(out=pool1[:, g, :, asl], in_=h1s,
                                        axis=mybir.AxisListType.X,
                                        op=Ad)
                nc.vector.tensor_reduce(out=pool2[:, g, :, asl], in_=h2,
                                        axis=mybir.AxisListType.X,
                                        op=Ad)

        # classifier over pooled [128=(u,h), 128=(g,j,a)]
        loop_ctx.close()
        pa = ppool.tile([128, G, 2, APG], F32, tag="pa")
        pb = ppool.tile([128, G, 2, APG], F32, tag="pb")
        nc.vector.tensor_scalar(out=pa, in0=pool1, scalar1=a1,
                                scalar2=None, op0=Mu)
        nc.vector.tensor_scalar(out=pb, in0=pool2, scalar1=a2,
                                scalar2=None, op0=Mu)
        pooled = ppool.tile([128, G, 2, APG], BF, tag="pooled")
        nc.vector.tensor_tensor(out=pooled, in0=pa, in1=pb, op=Ad)
        with tc.tile_pool(name="ps_cls", bufs=2, space="PSUM") as ps_cls:
            Rl = mybir.ActivationFunctionType.Relu
            for u in range(2):
                q1 = ps_cls.tile([128, 128], F32, tag="q1")
                nc.tensor.matmul(q1, w1_t[u * 64:(u + 1) * 64, :],
                                 pooled[u * 64:(u + 1) * 64, :, :, :],
                                 start=True, stop=True)
                r1 = ppool.tile([128, 128], BF, tag="r1")
                nc.scalar.activation(r1, q1, Rl)
                q2 = ps_cls.tile([1, G, 2, APG], F32, tag="q2")
                nc.tensor.matmul(q2, w2_t, r1, start=True, stop=True)
                # outbuf free order (g, a, u, j); view ordered (g, j, a)
                obv = bass.AP(tensor=outbuf.tensor,
                              offset=outbuf.offset + u * 2,
                              ap=[outbuf.ap[0], [64, G], [1, 2], [4, APG]])
                nc.vector.tensor_copy(out=obv, in_=q2)
        # out[s] with s = 64 g + 4 a + (2 u + j); src iter (g, a, u, j)
        base = out_d[:]
        out_ap = bass.AP(tensor=base.tensor, offset=base.offset,
                         ap=[[64, G], [4, APG], [1, 4]])
        nc.sync.dma_start(out=out_ap, in_=outbuf)
    return nc


_CACHE = {}


def _get_nc(a1, a2):
    key = (round(a1, 9), round(a2, 9), HOST_GATHER, PE_TRANSPOSE)
    if key not in _CACHE:
        _CACHE[key] = build(a1, a2)
    return _CACHE[key]


def kernel(_trace=False, **inputs):
    in_maps, a1, a2 = host_prep(inputs)
    nc = _get_nc(a1, a2)
    res = run_bass_kernel_spmd(nc, in_maps, core_ids=list(range(NCORES)),
                               trace=_trace)
    out = np.concatenate([res.results[i]["out"] for i in range(NCORES)])
    out = out.reshape(B, 1).astype(np.float32)
    if _trace:
        return out, res
    return out
